# revision 24
# baseline (speedup 1.0000x reference)
"""Trainium2 Bass kernel for PersonalizedSimpleAttention.

Computation (per batch b, person p = person_idxs[b]):
    keys    = x @ (PK_W[p] @ Wk).T + PK_b[p]               # folded projection
    queries = x @ (PQ_W[p] @ Wq / sqrt(KH)).T + PQ_b[p] / sqrt(KH)
    v       = x @ Wv.T
    attn    = softmax(queries @ keys.T + maskbias, axis=-1)
    out     = attn @ v                                     # [T, VH]

The personalized [KH,KH] @ [KH,EMB] fold happens on-device (4 small matmuls
per batch) and removes the k0/q0 intermediates of the naive two-stage form.

Sharding: data-parallel over batch across 8 cores (8 batches each); the
per-person weight stacks are gathered on the host (pure indexing) so each
core receives exactly its 8 weight matrices.  All on-device layouts are
transposed ([feature, token]) so every matmul contracts over the partition
dim with no on-device transposes; softmax runs over the partition (key) dim
via a pairwise DVE/GPSIMD adder tree + gpsimd partition_all_reduce, and the
normalization is folded in after the attn@v matmul (divide by denom once on
[KH, T] instead of on [T, T]).

Matmul operand dtype is selectable (bf16 default; f32r = TF32-like; f32)
with fp32 PSUM accumulation throughout.
"""
import math
import os

import numpy as np

import concourse.bass as bass  # noqa: F401  (registers engines)
import concourse.mybir as mybir
from concourse import bacc
from concourse.bass_utils import run_bass_kernel_spmd
from concourse.tile import TileContext

F32 = mybir.dt.float32
AF = mybir.ActivationFunctionType

B, T, EMB, KH = 64, 1024, 128, 256
NCORES = 8
BPC = B // NCORES          # batches per core
ST = T // 128              # 8 key tiles of 128
TB = T // 512              # 2 moving-dim blocks of 512

DT_MM_NAME = os.environ.get("BASS_KERNEL_DT", "bf16")
_DT_MAP = {"bf16": mybir.dt.bfloat16, "f32r": mybir.dt.float32r, "f32": F32}

_CACHE = {}


def _build_nc(dt_mm):
    nc = bacc.Bacc("TRN2", target_bir_lowering=False, debug=False)

    xT = nc.declare_dram_parameter("xT", [BPC, EMB, T], dt_mm, isOutput=False)
    wkn = nc.declare_dram_parameter("wkN", [KH, EMB], dt_mm, isOutput=False)
    wqn = nc.declare_dram_parameter("wqN", [KH, EMB], dt_mm, isOutput=False)
    wv = nc.declare_dram_parameter("wvT", [EMB, KH], dt_mm, isOutput=False)
    pkw = nc.declare_dram_parameter("pkwT", [BPC, KH, KH], dt_mm, isOutput=False)
    pqw = nc.declare_dram_parameter("pqwT", [BPC, KH, KH], dt_mm, isOutput=False)
    # aux = [pkb | pqb | mb] packed: [BPC, 2*KH + T]
    aux = nc.declare_dram_parameter("aux", [BPC, 2 * KH + T], F32, isOutput=False)
    out = nc.declare_dram_parameter("out", [BPC, T, KH], F32, isOutput=True)
    KH1 = KH + 1  # v gets a ones column: attn @ [v | 1] yields the softmax denom

    with TileContext(nc) as tc:
        with tc.tile_pool(name="const", bufs=1) as cpool, \
             tc.tile_pool(name="work", bufs=3) as wpool, \
             tc.tile_pool(name="big", bufs=1) as bpool, \
             tc.tile_pool(name="psa", bufs=2, space="PSUM") as psa, \
             tc.tile_pool(name="psd", bufs=2, space="PSUM") as psdp, \
             tc.tile_pool(name="psx", bufs=2, space="PSUM") as psxp:

            # Wk/Wq natural [h, e] as 2 h-tiles side by side; WvT [e, d]
            wknt = cpool.tile([128, 2 * EMB], dt_mm, name="wknt")
            wqnt = cpool.tile([128, 2 * EMB], dt_mm, name="wqnt")
            wvt = cpool.tile([128, KH], dt_mm, name="wvt")
            nc.sync.dma_start(out=wknt.rearrange("p (hh e) -> p hh e", hh=2),
                              in_=wkn.ap().rearrange("(hh p) e -> p hh e", p=128))
            nc.sync.dma_start(out=wqnt.rearrange("p (hh e) -> p hh e", hh=2),
                              in_=wqn.ap().rearrange("(hh p) e -> p hh e", p=128))
            nc.sync.dma_start(out=wvt, in_=wv[:, :])

            # ---- fold person matrices into projection weights (all batches
            # upfront, so the steady-state loop never waits on this chain) ---
            # weffT[e, o] = sum_h W[h, e] * PW_T[h, o]
            wkeffs = [cpool.tile([128, KH], dt_mm, name=f"wkeff{b}") for b in range(BPC)]
            wqeffs = [cpool.tile([128, KH], dt_mm, name=f"wqeff{b}") for b in range(BPC)]
            # batch 0's inputs go out first so its projections start early
            xt0 = wpool.tile([128, T], dt_mm, name="xt", tag="xt")
            nc.sync.dma_start(out=xt0, in_=xT[0])
            auxt0 = wpool.tile([128, 4 + ST], F32, name="auxt", tag="auxt")
            nc.sync.dma_start(out=auxt0, in_=aux[0].rearrange("(a p) -> p a", p=128))
            with tc.tile_pool(name="pw", bufs=1) as pwpool:
                pwk = pwpool.tile([128, BPC * 2 * KH], dt_mm, name="pwk")
                pwq = pwpool.tile([128, BPC * 2 * KH], dt_mm, name="pwq")
                nc.sync.dma_start(
                    out=pwk.rearrange("p (bb hh o) -> p bb hh o", bb=BPC, hh=2),
                    in_=pkw.ap().rearrange("bb (hh p) o -> p bb hh o", p=128))
                nc.gpsimd.dma_start(
                    out=pwq.rearrange("p (bb hh o) -> p bb hh o", bb=BPC, hh=2),
                    in_=pqw.ap().rearrange("bb (hh p) o -> p bb hh o", p=128))
                for b in range(BPC):
                    for (pwt, wn, weff) in ((pwk, wknt, wkeffs[b]), (pwq, wqnt, wqeffs[b])):
                        pse = psa.tile([128, KH], F32, name="pse", tag="a")
                        for hh in range(2):
                            nc.tensor.matmul(
                                pse, wn[:, hh * EMB:(hh + 1) * EMB],
                                pwt[:, (2 * b + hh) * KH:(2 * b + hh + 1) * KH],
                                start=(hh == 0), stop=(hh == 1))
                        nc.scalar.copy(weff, pse)

            for b in range(BPC):
                # ---- load per-batch operands -------------------------------
                if b == 0:
                    xt, auxt = xt0, auxt0
                else:
                    xt = wpool.tile([128, T], dt_mm, name="xt", tag="xt")
                    nc.sync.dma_start(out=xt, in_=xT[b])
                    auxt = wpool.tile([128, 4 + ST], F32, name="auxt", tag="auxt")
                    nc.sync.dma_start(out=auxt,
                                      in_=aux[b].rearrange("(a p) -> p a", p=128))
                pkbt = auxt[:, 0:2]
                pqbt = auxt[:, 2:4]
                mbt = auxt[:, 4:4 + ST]

                # ---- projections ------------------------------------------
                # keysT/queriesT: [o, s] as [128, oh*T + s]
                # v (with ones column): [s, d] as [128, st*KH1 + d], col KH = 1.0
                kt = wpool.tile([128, 2 * T], dt_mm, name="kt")
                qt = wpool.tile([128, 2 * T], dt_mm, name="qt")
                vt = wpool.tile([128, ST * KH1], dt_mm, name="vt")
                for (weff, bt_, dst) in ((wkeffs[b], pkbt, kt), (wqeffs[b], pqbt, qt)):
                    for oh in range(2):
                        for sb in range(TB):
                            psp = psa.tile([128, 512], F32, name="psp", tag="a")
                            nc.tensor.matmul(psp, weff[:, oh * 128:(oh + 1) * 128],
                                             xt[:, sb * 512:(sb + 1) * 512])
                            nc.vector.tensor_scalar_add(
                                dst[:, oh * T + sb * 512:oh * T + (sb + 1) * 512],
                                psp, bt_[:, oh:oh + 1])
                nc.vector.memset(
                    vt.rearrange("p (st c) -> p st c", c=KH1)[:, :, KH:KH1], 1.0)
                for st in range(ST):
                    psv = psa.tile([128, KH], F32, name="psv", tag="a")
                    nc.tensor.matmul(psv, xt[:, st * 128:(st + 1) * 128], wvt)
                    nc.scalar.copy(vt[:, st * KH1:st * KH1 + KH], psv)

                # ---- attention scores + exp -------------------------------
                # dotT: [s, t]; E[s, t] = exp(dot + maskbias[s])
                et = bpool.tile([128, ST * T], dt_mm, name="et", bufs=1)
                for st in range(ST):
                    psd = psdp.tile([128, T], F32, name="psd", tag="d")
                    for dh in range(2):
                        lhs = kt[:, dh * T + st * 128:dh * T + (st + 1) * 128]
                        for tb in range(TB):
                            nc.tensor.matmul(
                                psd[:, tb * 512:(tb + 1) * 512], lhs,
                                qt[:, dh * T + tb * 512:dh * T + (tb + 1) * 512],
                                start=(dh == 0), stop=(dh == 1))
                    nc.scalar.activation(et[:, st * T:(st + 1) * T], psd,
                                         AF.Exp, bias=mbt[:, st:st + 1])

                # ---- context + softmax denominator in one matmul ----------
                # ctx_aug[t, 0:KH]  = sum_s E[s, t] * v[s, d]
                # ctx_aug[t, KH]    = sum_s E[s, t]          (the ones column)
                # Output lands in natural [t, d] layout; normalization is a
                # per-partition reciprocal+scale, no transpose anywhere.
                for tt in range(ST):
                    psx = psxp.tile([128, KH1], F32, name="psx", tag="x")
                    for st in range(ST):
                        nc.tensor.matmul(
                            psx, et[:, st * T + tt * 128:st * T + (tt + 1) * 128],
                            vt[:, st * KH1:(st + 1) * KH1],
                            start=(st == 0), stop=(st == ST - 1))
                    rcpc = wpool.tile([128, 1], F32, name="rcpc")
                    nc.vector.reciprocal(rcpc, psx[:, KH:KH1])
                    ctxn = wpool.tile([128, KH], F32, name="ctxn")
                    nc.vector.tensor_scalar_mul(ctxn, psx[:, 0:KH], rcpc)
                    nc.sync.dma_start(out=out[b, tt * 128:(tt + 1) * 128, :],
                                      in_=ctxn)

    nc.compile()
    return nc


def _get_nc():
    if "nc" not in _CACHE:
        _CACHE["nc"] = _build_nc(_DT_MAP[DT_MM_NAME])
    return _CACHE["nc"]


def _np_mm_dtype():
    if DT_MM_NAME == "bf16":
        import ml_dtypes
        return np.dtype(ml_dtypes.bfloat16)
    return np.float32


def build_in_maps(x, mask, person_idxs, Wk, Wq, Wv, PK_W, PK_b, PQ_W, PQ_b):
    x = np.asarray(x, dtype=np.float32)
    mask = np.asarray(mask)
    idx = np.asarray(person_idxs).astype(np.int64)
    sk = 1.0 / math.sqrt(KH)
    mdt = _np_mm_dtype()

    wkN = np.ascontiguousarray(np.asarray(Wk, np.float32)).astype(mdt)   # [KH, EMB]
    wqN = np.ascontiguousarray(np.asarray(Wq, np.float32)).astype(mdt)
    wvT = np.ascontiguousarray(np.asarray(Wv, np.float32).T).astype(mdt)
    mbias = np.where(mask[:, 0, :], 0.0, -30.0).astype(np.float32)  # [B, T]

    in_maps = []
    for c in range(NCORES):
        bs = slice(c * BPC, (c + 1) * BPC)
        ci = idx[bs]
        in_maps.append({
            "xT": np.ascontiguousarray(x[bs].transpose(0, 2, 1)).astype(mdt),
            "wkN": wkN, "wqN": wqN, "wvT": wvT,
            "pkwT": np.ascontiguousarray(
                np.asarray(PK_W, np.float32)[ci].transpose(0, 2, 1)).astype(mdt),
            "pqwT": np.ascontiguousarray(
                (np.asarray(PQ_W, np.float32)[ci] * sk).transpose(0, 2, 1)).astype(mdt),
            "aux": np.ascontiguousarray(np.concatenate([
                np.asarray(PK_b, np.float32)[ci],
                np.asarray(PQ_b, np.float32)[ci] * sk,
                mbias[bs]], axis=1)),
        })
    return in_maps


def kernel(x, mask, person_idxs, Wk, Wq, Wv, PK_W, PK_b, PQ_W, PQ_b):
    in_maps = build_in_maps(x, mask, person_idxs, Wk, Wq, Wv, PK_W, PK_b, PQ_W, PQ_b)
    nc = _get_nc()
    res = run_bass_kernel_spmd(nc, in_maps, list(range(NCORES)))
    return np.concatenate([res.results[c]["out"] for c in range(NCORES)], axis=0)


# revision 25
# speedup vs baseline: 1.2021x; 1.2021x over previous
"""Trainium2 Bass kernel for PersonalizedSimpleAttention.

Computation (per batch b, person p = person_idxs[b]):
    keys    = x @ (PK_W[p] @ Wk).T + PK_b[p]               # folded projection
    queries = x @ (PQ_W[p] @ Wq / sqrt(KH)).T + PQ_b[p] / sqrt(KH)
    v       = x @ Wv.T
    attn    = softmax(queries @ keys.T + maskbias, axis=-1)
    out     = attn @ v                                     # [T, VH]

The personalized [KH,KH] @ [KH,EMB] fold happens on-device (4 small matmuls
per batch) and removes the k0/q0 intermediates of the naive two-stage form.

Sharding: data-parallel over batch across 8 cores (8 batches each); the
per-person weight stacks are gathered on the host (pure indexing) so each
core receives exactly its 8 weight matrices.  All on-device layouts are
transposed ([feature, token]) so every matmul contracts over the partition
dim with no on-device transposes; softmax runs over the partition (key) dim
via a pairwise DVE/GPSIMD adder tree + gpsimd partition_all_reduce, and the
normalization is folded in after the attn@v matmul (divide by denom once on
[KH, T] instead of on [T, T]).

Matmul operand dtype is selectable (bf16 default; f32r = TF32-like; f32)
with fp32 PSUM accumulation throughout.
"""
import math
import os

import numpy as np

import concourse.bass as bass  # noqa: F401  (registers engines)
import concourse.mybir as mybir
from concourse import bacc
from concourse.bass_utils import run_bass_kernel_spmd
from concourse.tile import TileContext

F32 = mybir.dt.float32
AF = mybir.ActivationFunctionType

B, T, EMB, KH = 64, 1024, 128, 256
NCORES = 8
BPC = B // NCORES          # batches per core
ST = T // 128              # 8 key tiles of 128
TB = T // 512              # 2 moving-dim blocks of 512

DT_MM_NAME = os.environ.get("BASS_KERNEL_DT", "bf16")
_DT_MAP = {"bf16": mybir.dt.bfloat16, "f32r": mybir.dt.float32r, "f32": F32}

_CACHE = {}


def _build_nc(dt_mm):
    nc = bacc.Bacc("TRN2", target_bir_lowering=False, debug=False)

    xT = nc.declare_dram_parameter("xT", [BPC, EMB, T], dt_mm, isOutput=False)
    wkn = nc.declare_dram_parameter("wkN", [KH, EMB], dt_mm, isOutput=False)
    wqn = nc.declare_dram_parameter("wqN", [KH, EMB], dt_mm, isOutput=False)
    wv = nc.declare_dram_parameter("wvT", [EMB, KH], dt_mm, isOutput=False)
    pkw = nc.declare_dram_parameter("pkwT", [BPC, KH, KH], dt_mm, isOutput=False)
    pqw = nc.declare_dram_parameter("pqwT", [BPC, KH, KH], dt_mm, isOutput=False)
    # aux = [pkb | pqb | mb] packed: [BPC, 2*KH + T]
    aux = nc.declare_dram_parameter("aux", [BPC, 2 * KH + T], F32, isOutput=False)
    out = nc.declare_dram_parameter("out", [BPC, T, KH], F32, isOutput=True)
    KH1 = KH + 1  # v gets a ones column: attn @ [v | 1] yields the softmax denom

    with TileContext(nc) as tc:
        with tc.tile_pool(name="const", bufs=1) as cpool, \
             tc.tile_pool(name="work", bufs=3) as wpool, \
             tc.tile_pool(name="big", bufs=1) as bpool, \
             tc.tile_pool(name="psa", bufs=2, space="PSUM") as psa, \
             tc.tile_pool(name="psd", bufs=2, space="PSUM") as psdp, \
             tc.tile_pool(name="psx", bufs=2, space="PSUM") as psxp:

            # Wk/Wq natural [h, e] as 2 h-tiles side by side; WvT [e, d]
            wknt = cpool.tile([128, 2 * EMB], dt_mm, name="wknt")
            wqnt = cpool.tile([128, 2 * EMB], dt_mm, name="wqnt")
            wvt = cpool.tile([128, KH], dt_mm, name="wvt")
            nc.sync.dma_start(out=wknt.rearrange("p (hh e) -> p hh e", hh=2),
                              in_=wkn.ap().rearrange("(hh p) e -> p hh e", p=128))
            nc.sync.dma_start(out=wqnt.rearrange("p (hh e) -> p hh e", hh=2),
                              in_=wqn.ap().rearrange("(hh p) e -> p hh e", p=128))
            nc.sync.dma_start(out=wvt, in_=wv[:, :])

            # ---- fold person matrices into projection weights (all batches
            # upfront, so the steady-state loop never waits on this chain) ---
            # weffT[e, o] = sum_h W[h, e] * PW_T[h, o]
            wkeffs = [cpool.tile([128, KH], dt_mm, name=f"wkeff{b}") for b in range(BPC)]
            wqeffs = [cpool.tile([128, KH], dt_mm, name=f"wqeff{b}") for b in range(BPC)]
            # batch 0's inputs go out first so its projections start early
            xt0 = wpool.tile([128, T], dt_mm, name="xt", tag="xt")
            nc.sync.dma_start(out=xt0, in_=xT[0])
            auxt0 = wpool.tile([128, 4 + ST], F32, name="auxt", tag="auxt")
            nc.sync.dma_start(out=auxt0, in_=aux[0].rearrange("(a p) -> p a", p=128))
            with tc.tile_pool(name="pw", bufs=16) as pwpool:
                pwts = []
                for b in range(BPC):
                    for (j, pw_d) in ((0, pkw), (1, pqw)):
                        pwt = pwpool.tile([128, 2 * KH], dt_mm, name=f"pwt{b}_{j}", tag="pwt")
                        eng = nc.sync if (2 * b + j) % 2 == 0 else nc.gpsimd
                        eng.dma_start(
                            out=pwt.rearrange("p (hh o) -> p hh o", hh=2),
                            in_=pw_d[b].rearrange("(hh p) o -> p hh o", p=128))
                        pwts.append(pwt)
                for b in range(BPC):
                    for (j, wn, weff) in ((0, wknt, wkeffs[b]), (1, wqnt, wqeffs[b])):
                        pwt = pwts[2 * b + j]
                        pse = psa.tile([128, KH], F32, name="pse", tag="a")
                        for hh in range(2):
                            nc.tensor.matmul(pse, wn[:, hh * EMB:(hh + 1) * EMB],
                                             pwt[:, hh * KH:(hh + 1) * KH],
                                             start=(hh == 0), stop=(hh == 1))
                        nc.scalar.copy(weff, pse)

            for b in range(BPC):
                # ---- load per-batch operands -------------------------------
                if b == 0:
                    xt, auxt = xt0, auxt0
                else:
                    xt = wpool.tile([128, T], dt_mm, name="xt", tag="xt")
                    nc.sync.dma_start(out=xt, in_=xT[b])
                    auxt = wpool.tile([128, 4 + ST], F32, name="auxt", tag="auxt")
                    nc.sync.dma_start(out=auxt,
                                      in_=aux[b].rearrange("(a p) -> p a", p=128))
                pkbt = auxt[:, 0:2]
                pqbt = auxt[:, 2:4]
                mbt = auxt[:, 4:4 + ST]

                # ---- projections ------------------------------------------
                # keysT/queriesT: [o, s] as [128, oh*T + s]
                # v (with ones column): [s, d] as [128, st*KH1 + d], col KH = 1.0
                kt = wpool.tile([128, 2 * T], dt_mm, name="kt")
                qt = wpool.tile([128, 2 * T], dt_mm, name="qt")
                vt = wpool.tile([128, ST * KH1], dt_mm, name="vt")
                for (weff, bt_, dst) in ((wkeffs[b], pkbt, kt), (wqeffs[b], pqbt, qt)):
                    for oh in range(2):
                        for sb in range(TB):
                            psp = psa.tile([128, 512], F32, name="psp", tag="a")
                            nc.tensor.matmul(psp, weff[:, oh * 128:(oh + 1) * 128],
                                             xt[:, sb * 512:(sb + 1) * 512])
                            nc.vector.tensor_scalar_add(
                                dst[:, oh * T + sb * 512:oh * T + (sb + 1) * 512],
                                psp, bt_[:, oh:oh + 1])
                nc.vector.memset(
                    vt.rearrange("p (st c) -> p st c", c=KH1)[:, :, KH:KH1], 1.0)
                for st in range(ST):
                    psv = psa.tile([128, KH], F32, name="psv", tag="a")
                    nc.tensor.matmul(psv, xt[:, st * 128:(st + 1) * 128], wvt)
                    nc.scalar.copy(vt[:, st * KH1:st * KH1 + KH], psv)

                # ---- attention scores + exp -------------------------------
                # dotT: [s, t]; E[s, t] = exp(dot + maskbias[s])
                et = bpool.tile([128, ST * T], dt_mm, name="et", bufs=1)
                for st in range(ST):
                    psd = psdp.tile([128, T], F32, name="psd", tag="d")
                    for dh in range(2):
                        lhs = kt[:, dh * T + st * 128:dh * T + (st + 1) * 128]
                        for tb in range(TB):
                            nc.tensor.matmul(
                                psd[:, tb * 512:(tb + 1) * 512], lhs,
                                qt[:, dh * T + tb * 512:dh * T + (tb + 1) * 512],
                                start=(dh == 0), stop=(dh == 1))
                    nc.scalar.activation(et[:, st * T:(st + 1) * T], psd,
                                         AF.Exp, bias=mbt[:, st:st + 1])

                # ---- context + softmax denominator in one matmul ----------
                # ctx_aug[t, 0:KH]  = sum_s E[s, t] * v[s, d]
                # ctx_aug[t, KH]    = sum_s E[s, t]          (the ones column)
                # Output lands in natural [t, d] layout; normalization is a
                # per-partition reciprocal+scale, no transpose anywhere.
                for tt in range(ST):
                    psx = psxp.tile([128, KH1], F32, name="psx", tag="x")
                    for st in range(ST):
                        nc.tensor.matmul(
                            psx, et[:, st * T + tt * 128:st * T + (tt + 1) * 128],
                            vt[:, st * KH1:(st + 1) * KH1],
                            start=(st == 0), stop=(st == ST - 1))
                    rcpc = wpool.tile([128, 1], F32, name="rcpc")
                    nc.vector.reciprocal(rcpc, psx[:, KH:KH1])
                    ctxn = wpool.tile([128, KH], F32, name="ctxn")
                    nc.vector.tensor_scalar_mul(ctxn, psx[:, 0:KH], rcpc)
                    nc.sync.dma_start(out=out[b, tt * 128:(tt + 1) * 128, :],
                                      in_=ctxn)

    nc.compile()
    return nc


def _get_nc():
    if "nc" not in _CACHE:
        _CACHE["nc"] = _build_nc(_DT_MAP[DT_MM_NAME])
    return _CACHE["nc"]


def _np_mm_dtype():
    if DT_MM_NAME == "bf16":
        import ml_dtypes
        return np.dtype(ml_dtypes.bfloat16)
    return np.float32


def build_in_maps(x, mask, person_idxs, Wk, Wq, Wv, PK_W, PK_b, PQ_W, PQ_b):
    x = np.asarray(x, dtype=np.float32)
    mask = np.asarray(mask)
    idx = np.asarray(person_idxs).astype(np.int64)
    sk = 1.0 / math.sqrt(KH)
    mdt = _np_mm_dtype()

    wkN = np.ascontiguousarray(np.asarray(Wk, np.float32)).astype(mdt)   # [KH, EMB]
    wqN = np.ascontiguousarray(np.asarray(Wq, np.float32)).astype(mdt)
    wvT = np.ascontiguousarray(np.asarray(Wv, np.float32).T).astype(mdt)
    mbias = np.where(mask[:, 0, :], 0.0, -30.0).astype(np.float32)  # [B, T]

    in_maps = []
    for c in range(NCORES):
        bs = slice(c * BPC, (c + 1) * BPC)
        ci = idx[bs]
        in_maps.append({
            "xT": np.ascontiguousarray(x[bs].transpose(0, 2, 1)).astype(mdt),
            "wkN": wkN, "wqN": wqN, "wvT": wvT,
            "pkwT": np.ascontiguousarray(
                np.asarray(PK_W, np.float32)[ci].transpose(0, 2, 1)).astype(mdt),
            "pqwT": np.ascontiguousarray(
                (np.asarray(PQ_W, np.float32)[ci] * sk).transpose(0, 2, 1)).astype(mdt),
            "aux": np.ascontiguousarray(np.concatenate([
                np.asarray(PK_b, np.float32)[ci],
                np.asarray(PQ_b, np.float32)[ci] * sk,
                mbias[bs]], axis=1)),
        })
    return in_maps


def kernel(x, mask, person_idxs, Wk, Wq, Wv, PK_W, PK_b, PQ_W, PQ_b):
    in_maps = build_in_maps(x, mask, person_idxs, Wk, Wq, Wv, PK_W, PK_b, PQ_W, PQ_b)
    nc = _get_nc()
    res = run_bass_kernel_spmd(nc, in_maps, list(range(NCORES)))
    return np.concatenate([res.results[c]["out"] for c in range(NCORES)], axis=0)


# revision 26
# speedup vs baseline: 1.3001x; 1.0815x over previous
"""Trainium2 Bass kernel for PersonalizedSimpleAttention.

Computation (per batch b, person p = person_idxs[b]):
    keys    = x @ (PK_W[p] @ Wk).T + PK_b[p]               # folded projection
    queries = x @ (PQ_W[p] @ Wq / sqrt(KH)).T + PQ_b[p] / sqrt(KH)
    v       = x @ Wv.T
    attn    = softmax(queries @ keys.T + maskbias, axis=-1)
    out     = attn @ v                                     # [T, VH]

The personalized [KH,KH] @ [KH,EMB] fold happens on-device (4 small matmuls
per batch) and removes the k0/q0 intermediates of the naive two-stage form.

Sharding: data-parallel over batch across 8 cores (8 batches each); the
per-person weight stacks are gathered on the host (pure indexing) so each
core receives exactly its 8 weight matrices.  All on-device layouts are
transposed ([feature, token]) so every matmul contracts over the partition
dim with no on-device transposes; softmax runs over the partition (key) dim
via a pairwise DVE/GPSIMD adder tree + gpsimd partition_all_reduce, and the
normalization is folded in after the attn@v matmul (divide by denom once on
[KH, T] instead of on [T, T]).

Matmul operand dtype is selectable (bf16 default; f32r = TF32-like; f32)
with fp32 PSUM accumulation throughout.
"""
import math
import os

import numpy as np

import concourse.bass as bass  # noqa: F401  (registers engines)
import concourse.mybir as mybir
from concourse import bacc
from concourse.bass_utils import run_bass_kernel_spmd
from concourse.tile import TileContext

F32 = mybir.dt.float32
AF = mybir.ActivationFunctionType

B, T, EMB, KH = 64, 1024, 128, 256
NCORES = 8
BPC = B // NCORES          # batches per core
ST = T // 128              # 8 key tiles of 128
TB = T // 512              # 2 moving-dim blocks of 512

DT_MM_NAME = os.environ.get("BASS_KERNEL_DT", "bf16")
_DT_MAP = {"bf16": mybir.dt.bfloat16, "f32r": mybir.dt.float32r, "f32": F32}

_CACHE = {}


def _build_nc(dt_mm):
    nc = bacc.Bacc("TRN2", target_bir_lowering=False, debug=False)

    xT = nc.declare_dram_parameter("xT", [BPC, EMB, T], dt_mm, isOutput=False)
    wkn = nc.declare_dram_parameter("wkN", [KH, EMB], dt_mm, isOutput=False)
    wqn = nc.declare_dram_parameter("wqN", [KH, EMB], dt_mm, isOutput=False)
    wv = nc.declare_dram_parameter("wvT", [EMB, KH], dt_mm, isOutput=False)
    pkw = nc.declare_dram_parameter("pkwT", [BPC, KH, KH], dt_mm, isOutput=False)
    pqw = nc.declare_dram_parameter("pqwT", [BPC, KH, KH], dt_mm, isOutput=False)
    # aux = [pkb | pqb | mb] packed: [BPC, 2*KH + T]
    aux = nc.declare_dram_parameter("aux", [BPC, 2 * KH + T], F32, isOutput=False)
    out = nc.declare_dram_parameter("out", [BPC, T, KH], F32, isOutput=True)
    KH1 = KH + 1  # v gets a ones column: attn @ [v | 1] yields the softmax denom

    with TileContext(nc) as tc:
        with tc.tile_pool(name="const", bufs=1) as cpool, \
             tc.tile_pool(name="work", bufs=3) as wpool, \
             tc.tile_pool(name="big", bufs=1) as bpool, \
             tc.tile_pool(name="psa", bufs=3, space="PSUM") as psa, \
             tc.tile_pool(name="psd", bufs=3, space="PSUM") as psdp, \
             tc.tile_pool(name="psx", bufs=2, space="PSUM") as psxp:

            # Wk/Wq natural [h, e] as 2 h-tiles side by side; WvT [e, d]
            wknt = cpool.tile([128, 2 * EMB], dt_mm, name="wknt")
            wqnt = cpool.tile([128, 2 * EMB], dt_mm, name="wqnt")
            wvt = cpool.tile([128, KH], dt_mm, name="wvt")
            nc.sync.dma_start(out=wknt.rearrange("p (hh e) -> p hh e", hh=2),
                              in_=wkn.ap().rearrange("(hh p) e -> p hh e", p=128))
            nc.sync.dma_start(out=wqnt.rearrange("p (hh e) -> p hh e", hh=2),
                              in_=wqn.ap().rearrange("(hh p) e -> p hh e", p=128))
            nc.sync.dma_start(out=wvt, in_=wv[:, :])

            # ---- fold person matrices into projection weights (all batches
            # upfront, so the steady-state loop never waits on this chain) ---
            # weffT[e, o] = sum_h W[h, e] * PW_T[h, o]
            wkeffs = [cpool.tile([128, KH], dt_mm, name=f"wkeff{b}") for b in range(BPC)]
            wqeffs = [cpool.tile([128, KH], dt_mm, name=f"wqeff{b}") for b in range(BPC)]
            # batch 0's inputs go out first so its projections start early
            xt0 = wpool.tile([128, T], dt_mm, name="xt", tag="xt")
            nc.sync.dma_start(out=xt0, in_=xT[0])
            auxt0 = wpool.tile([128, 4 + ST], F32, name="auxt", tag="auxt")
            nc.sync.dma_start(out=auxt0, in_=aux[0].rearrange("(a p) -> p a", p=128))
            with tc.tile_pool(name="pw", bufs=16) as pwpool:
                pwts = []
                for b in range(BPC):
                    for (j, pw_d) in ((0, pkw), (1, pqw)):
                        pwt = pwpool.tile([128, 2 * KH], dt_mm, name=f"pwt{b}_{j}", tag="pwt")
                        eng = nc.sync if (2 * b + j) % 2 == 0 else nc.gpsimd
                        eng.dma_start(
                            out=pwt.rearrange("p (hh o) -> p hh o", hh=2),
                            in_=pw_d[b].rearrange("(hh p) o -> p hh o", p=128))
                        pwts.append(pwt)
                for b in range(BPC):
                    for (j, wn, weff) in ((0, wknt, wkeffs[b]), (1, wqnt, wqeffs[b])):
                        pwt = pwts[2 * b + j]
                        pse = psa.tile([128, KH], F32, name="pse", tag="a")
                        for hh in range(2):
                            nc.tensor.matmul(pse, wn[:, hh * EMB:(hh + 1) * EMB],
                                             pwt[:, hh * KH:(hh + 1) * KH],
                                             start=(hh == 0), stop=(hh == 1))
                        nc.scalar.copy(weff, pse)

            for b in range(BPC):
                # ---- load per-batch operands -------------------------------
                if b == 0:
                    xt, auxt = xt0, auxt0
                else:
                    xt = wpool.tile([128, T], dt_mm, name="xt", tag="xt")
                    nc.sync.dma_start(out=xt, in_=xT[b])
                    auxt = wpool.tile([128, 4 + ST], F32, name="auxt", tag="auxt")
                    nc.sync.dma_start(out=auxt,
                                      in_=aux[b].rearrange("(a p) -> p a", p=128))
                pkbt = auxt[:, 0:2]
                pqbt = auxt[:, 2:4]
                mbt = auxt[:, 4:4 + ST]

                # ---- projections ------------------------------------------
                # keysT/queriesT: [o, s] as [128, oh*T + s]
                # v (with ones column): [s, d] as [128, st*KH1 + d], col KH = 1.0
                kt = wpool.tile([128, 2 * T], dt_mm, name="kt")
                qt = wpool.tile([128, 2 * T], dt_mm, name="qt")
                vt = wpool.tile([128, ST * KH1], dt_mm, name="vt")
                for (weff, bt_, dst) in ((wkeffs[b], pkbt, kt), (wqeffs[b], pqbt, qt)):
                    for oh in range(2):
                        for sb in range(TB):
                            psp = psa.tile([128, 512], F32, name="psp", tag="a")
                            nc.tensor.matmul(psp, weff[:, oh * 128:(oh + 1) * 128],
                                             xt[:, sb * 512:(sb + 1) * 512])
                            nc.vector.tensor_scalar_add(
                                dst[:, oh * T + sb * 512:oh * T + (sb + 1) * 512],
                                psp, bt_[:, oh:oh + 1])
                nc.vector.memset(
                    vt.rearrange("p (st c) -> p st c", c=KH1)[:, :, KH:KH1], 1.0)
                for st in range(ST):
                    psv = psa.tile([128, KH], F32, name="psv", tag="a")
                    nc.tensor.matmul(psv, xt[:, st * 128:(st + 1) * 128], wvt)
                    nc.scalar.copy(vt[:, st * KH1:st * KH1 + KH], psv)

                # ---- attention scores + exp -------------------------------
                # dotT: [s, t]; E[s, t] = exp(dot + maskbias[s])
                et = bpool.tile([128, ST * T], dt_mm, name="et", bufs=1)
                for st in range(ST):
                    psd = [psdp.tile([128, 512], F32, name=f"psd{tb}", tag="d")
                           for tb in range(TB)]
                    for dh in range(2):
                        lhs = kt[:, dh * T + st * 128:dh * T + (st + 1) * 128]
                        for tb in range(TB):
                            nc.tensor.matmul(
                                psd[tb], lhs,
                                qt[:, dh * T + tb * 512:dh * T + (tb + 1) * 512],
                                start=(dh == 0), stop=(dh == 1))
                    for tb in range(TB):
                        nc.scalar.activation(
                            et[:, st * T + tb * 512:st * T + (tb + 1) * 512],
                            psd[tb], AF.Exp, bias=mbt[:, st:st + 1])

                # ---- context + softmax denominator in one matmul ----------
                # ctx_aug[t, 0:KH]  = sum_s E[s, t] * v[s, d]
                # ctx_aug[t, KH]    = sum_s E[s, t]          (the ones column)
                # Output lands in natural [t, d] layout; normalization is a
                # per-partition reciprocal+scale, no transpose anywhere.
                for tt in range(ST):
                    psx = psxp.tile([128, KH1], F32, name="psx", tag="x")
                    for st in range(ST):
                        nc.tensor.matmul(
                            psx, et[:, st * T + tt * 128:st * T + (tt + 1) * 128],
                            vt[:, st * KH1:(st + 1) * KH1],
                            start=(st == 0), stop=(st == ST - 1))
                    rcpc = wpool.tile([128, 1], F32, name="rcpc")
                    nc.vector.reciprocal(rcpc, psx[:, KH:KH1])
                    ctxn = wpool.tile([128, KH], F32, name="ctxn")
                    nc.vector.tensor_scalar_mul(ctxn, psx[:, 0:KH], rcpc)
                    nc.sync.dma_start(out=out[b, tt * 128:(tt + 1) * 128, :],
                                      in_=ctxn)

    nc.compile()
    return nc


def _get_nc():
    if "nc" not in _CACHE:
        _CACHE["nc"] = _build_nc(_DT_MAP[DT_MM_NAME])
    return _CACHE["nc"]


def _np_mm_dtype():
    if DT_MM_NAME == "bf16":
        import ml_dtypes
        return np.dtype(ml_dtypes.bfloat16)
    return np.float32


def build_in_maps(x, mask, person_idxs, Wk, Wq, Wv, PK_W, PK_b, PQ_W, PQ_b):
    x = np.asarray(x, dtype=np.float32)
    mask = np.asarray(mask)
    idx = np.asarray(person_idxs).astype(np.int64)
    sk = 1.0 / math.sqrt(KH)
    mdt = _np_mm_dtype()

    wkN = np.ascontiguousarray(np.asarray(Wk, np.float32)).astype(mdt)   # [KH, EMB]
    wqN = np.ascontiguousarray(np.asarray(Wq, np.float32)).astype(mdt)
    wvT = np.ascontiguousarray(np.asarray(Wv, np.float32).T).astype(mdt)
    mbias = np.where(mask[:, 0, :], 0.0, -30.0).astype(np.float32)  # [B, T]

    in_maps = []
    for c in range(NCORES):
        bs = slice(c * BPC, (c + 1) * BPC)
        ci = idx[bs]
        in_maps.append({
            "xT": np.ascontiguousarray(x[bs].transpose(0, 2, 1)).astype(mdt),
            "wkN": wkN, "wqN": wqN, "wvT": wvT,
            "pkwT": np.ascontiguousarray(
                np.asarray(PK_W, np.float32)[ci].transpose(0, 2, 1)).astype(mdt),
            "pqwT": np.ascontiguousarray(
                (np.asarray(PQ_W, np.float32)[ci] * sk).transpose(0, 2, 1)).astype(mdt),
            "aux": np.ascontiguousarray(np.concatenate([
                np.asarray(PK_b, np.float32)[ci],
                np.asarray(PQ_b, np.float32)[ci] * sk,
                mbias[bs]], axis=1)),
        })
    return in_maps


def kernel(x, mask, person_idxs, Wk, Wq, Wv, PK_W, PK_b, PQ_W, PQ_b):
    in_maps = build_in_maps(x, mask, person_idxs, Wk, Wq, Wv, PK_W, PK_b, PQ_W, PQ_b)
    nc = _get_nc()
    res = run_bass_kernel_spmd(nc, in_maps, list(range(NCORES)))
    return np.concatenate([res.results[c]["out"] for c in range(NCORES)], axis=0)
